# revision 50
# baseline (speedup 1.0000x reference)
"""AA_GAT on 8 trn2 cores (self-contained), v2.

Three launches; host does layout/gather only between launches.

L1: node MLP (nodes sharded 1/8 per core) + edge MLP (edges sharded by
    src-window). LN via Cholesky trick: y' = centered pre-LN output and
    u-columns come out of one matmul; var = sum(u^2)/64. beta=0 lets
    relu commute with the 1/sigma scale, so the only PSUM->SBUF bridge
    is a plain Relu; the iv scale is applied to the tiny outputs
    (adots 16 cols, esc 9 cols, x 64 cols once per node tile).
L2: layer-1 8-head edge pass per src window. Scores summed on PE from
    a host-transposed component table (esc8|asrc8|atgt8); exp on Act;
    per-edge value weighting V = w8 (x) xg via three engine paths
    (Act-replicate + DVE-stt / DVE tensor_tensor / Pool stt), one-hot
    segment-sum matmuls (host-prebuilt S3), elu(elu(.)), out-layer
    h_out = xh @ out_W + alpha dots.
L3: out-layer edge pass (same slot layout), then batched log_softmax.
"""

import numpy as np

import concourse.bass as bass
import concourse.mybir as mybir
import concourse.tile as tile
from concourse import bacc
from concourse.bass_utils import run_bass_kernel_spmd
from concourse.masks import make_identity

F32 = mybir.dt.float32
BF16 = mybir.dt.bfloat16
AF = mybir.ActivationFunctionType
OP = mybir.AluOpType
AX = mybir.AxisListType

N = 50000
E = 1_000_000
NODE_DIM = 16
EMB = 64
OUT = 64
HEADS = 8
EA_DIM = 8
SLOPE = 0.01
LN_EPS = 1e-5

NCORES = 8
NWIN = 49                 # windows (128 src nodes) per core
NPC = NWIN * 128          # 6272 nodes per core
NPN = NCORES * NPC        # 50176 padded node count
NWTOT = NCORES * NWIN     # 392 windows total
NDMA3 = 8                 # launch-3 one-hot chunks loaded via DMA

# L2 per-chunk V-path assignment (tuned): 'B' Act-replicate + DVE stt,
# 'A' DVE tensor_tensor broadcast, 'C' Pool stt broadcast.


def _vpaths(KT):
    # A = Act double-replicate + packed DVE TT (2x), D = DVE TT broadcast,
    # P = Pool TT broadcast; 2:11:7 split
    order = "DPDPADDPDPDPDPADDPDP" * 4
    return [order[c % len(order)] for c in range(KT)]


# ------------------------------------------------------------------ host prep


def _prep(edge_index):
    """Degree-balanced node->window permutation and edge slot layout."""
    src = np.asarray(edge_index[0]).astype(np.int64)
    tgt = np.asarray(edge_index[1]).astype(np.int64)

    deg = np.bincount(src, minlength=N).astype(np.int64)
    # greedy: big-degree nodes first, into least-loaded window with space
    order = np.argsort(-deg, kind="stable")
    wload = np.zeros(NWTOT, np.int64)
    wcnt = np.zeros(NWTOT, np.int64)
    wnodes = [[] for _ in range(NWTOT)]
    import heapq

    heap = [(0, 0, w) for w in range(NWTOT)]
    heapq.heapify(heap)
    for n in order:
        while True:
            load, cnt, w = heapq.heappop(heap)
            if wcnt[w] < 128:
                break
        wnodes[w].append(n)
        wload[w] += deg[n]
        wcnt[w] += 1
        if wcnt[w] < 128:
            heapq.heappush(heap, (wload[w], wcnt[w], w))
    # order windows by load, snake-assign to cores for balance
    worder = np.argsort(-wload, kind="stable")
    core_wins = [[] for _ in range(NCORES)]
    fwd = True
    i = 0
    while i < NWTOT:
        rng = range(NCORES) if fwd else range(NCORES - 1, -1, -1)
        for k in rng:
            if i < NWTOT:
                core_wins[k].append(worder[i])
                i += 1
        fwd = not fwd
    # global permuted row id: core k, local window j, slot s
    pnode = np.full(N, -1, np.int64)
    origin = np.full(NPN, -1, np.int64)
    for k in range(NCORES):
        for j, w in enumerate(core_wins[k]):
            base = k * NPC + j * 128
            nodes = wnodes[w]
            for s, n in enumerate(nodes):
                pnode[n] = base + s
                origin[base + s] = n
    assert (pnode >= 0).all()

    psrc = pnode[src]
    ptgt = pnode[tgt]
    core_of = psrc // NPC
    win_of = (psrc % NPC) // 128
    srcw_of = psrc % 128

    KT = 0
    buckets = {}
    for k in range(NCORES):
        mk = core_of == k
        idx_k = np.nonzero(mk)[0]
        w = win_of[idx_k]
        for ww in range(NWIN):
            el = idx_k[w == ww]
            buckets[(k, ww)] = el
            KT = max(KT, (len(el) + 127) // 128)
    NS = KT * 128          # slots per window

    per_core = []
    for k in range(NCORES):
        eslot = np.full((NWIN, NS), -1, np.int64)     # edge id per slot
        for ww in range(NWIN):
            el = buckets[(k, ww)]
            eslot[ww, : len(el)] = el
        per_core.append(eslot)
    return per_core, pnode, origin, KT, srcw_of, ptgt


# ------------------------------------------------------------------ launch 1


def _build_launch1(NCHE, skip_node=False, max_blk=None, stop_at=99):
    """Node MLP (49 tiles) + edge MLP (NCHE chunks)."""
    nc = bacc.Bacc("TRN2", target_bir_lowering=False, debug=False,
                   num_devices=NCORES)
    din = lambda n, s, d=F32: nc.dram_tensor(n, s, d, kind="ExternalInput")
    XT17 = din("XT17", [NODE_DIM + 1, NPC], BF16)
    WNC = din("WNC", [NODE_DIM + 1, EMB + NODE_DIM + 1], BF16)
    WAB = din("WAB", [EMB, 16], BF16)
    AE9 = din("AE9", [128, 2, 16], BF16)     # [AE9;0] and [0;AE9] halves
    EAT9 = din("EAT9", [EA_DIM + 1, NCHE * 128], BF16)
    WEC = din("WEC", [EA_DIM + 1, EMB + EA_DIM + 1], BF16)

    XO = nc.dram_tensor("XO", [NPC, EMB], BF16, kind="ExternalOutput")
    AD = nc.dram_tensor("AD", [NPC, 16], F32, kind="ExternalOutput")
    ESC9 = nc.dram_tensor("ESC9", [128, NCHE, 9], F32, kind="ExternalOutput")

    KN = NODE_DIM + 1   # 17 u-cols (node)
    KE = EA_DIM + 1     # 9 u-cols (edge)

    with tile.TileContext(nc) as tc:
        with tc.tile_pool(name="const", bufs=1) as cpool:
            ident = cpool.tile([128, 128], BF16)
            make_identity(nc, ident[:])
            epst = cpool.tile([128, 1], F32)
            nc.gpsimd.memset(epst[:], LN_EPS)
            wnc_sb = cpool.tile([KN, EMB + KN], BF16)
            nc.sync.dma_start(wnc_sb[:], WNC[:])
            wab_sb = cpool.tile([EMB, 16], BF16)
            nc.sync.dma_start(wab_sb[:], WAB[:])
            ae9_sb = cpool.tile([128, 2, 16], BF16)
            nc.sync.dma_start(ae9_sb[:], AE9[:])
            wec_sb = cpool.tile([KE, EMB + KE], BF16)
            nc.sync.dma_start(wec_sb[:], WEC[:])

            # ------------- node MLP: 49 tiles, batch 4 for stats
            xout = cpool.tile([128, NWIN, EMB], BF16)
            adout = cpool.tile([128, NWIN, 16], F32)
            xt17 = cpool.tile([KN, NPC], BF16)
            nc.sync.dma_start(xt17[:], XT17[:])
            NG = 0 if skip_node else (NWIN + 3) // 4
            with (
                tc.tile_pool(name="na", bufs=3) as na,
                tc.tile_pool(name="na_ps", bufs=2, space="PSUM") as nap,
                tc.tile_pool(name="nt_ps", bufs=2, space="PSUM") as ntp,
                tc.tile_pool(name="nad_ps", bufs=2, space="PSUM") as nadp,
            ):
                for g in range(NG):
                    t0 = 4 * g
                    nt = min(4, NWIN - t0)
                    y4 = nap.tile([128, 4, EMB + KN], F32, tag="y4")
                    for t in range(nt):
                        nc.tensor.matmul(
                            y4[:, t, :],
                            lhsT=xt17[:, 128 * (t0 + t) : 128 * (t0 + t + 1)],
                            rhs=wnc_sb[:], start=True, stop=True)
                    u2 = na.tile([128, 4, KN], F32, tag="u2")
                    nc.scalar.activation(u2[:, :nt, :], y4[:, :nt, EMB:],
                                         AF.Square)
                    q = na.tile([128, 4], F32, tag="q")
                    nc.vector.tensor_reduce(q[:, :nt], u2[:, :nt, :],
                                            axis=AX.X, op=OP.add)
                    iv = na.tile([128, 4], F32, tag="iv")
                    nc.scalar.activation(iv[:, :nt], q[:, :nt], AF.Sqrt,
                                         bias=epst[:])
                    nc.vector.reciprocal(iv[:, :nt], iv[:, :nt])
                    for t in range(nt):
                        # x = max(iv*y'g, 0) directly into the table row
                        nc.vector.tensor_scalar(
                            xout[:, t0 + t, :], y4[:, t, :EMB],
                            iv[:, t : t + 1], 0.0, op0=OP.mult, op1=OP.max)
                        rT_ps = ntp.tile([EMB, 128], BF16, tag="rT")
                        nc.tensor.transpose(out=rT_ps[:],
                                            in_=xout[:, t0 + t, :],
                                            identity=ident[:])
                        rT = na.tile([EMB, 128], BF16, tag="rTs")
                        nc.scalar.activation(rT[:], rT_ps[:], AF.Copy)
                        a_ps = nadp.tile([128, 16], F32, tag="aps")
                        nc.tensor.matmul(a_ps[:], lhsT=rT[:], rhs=wab_sb[:],
                                         start=True, stop=True)
                        nc.vector.tensor_scalar(
                            adout[:, t0 + t, :], a_ps[:], 1.0, None,
                            op0=OP.mult)
            if skip_node:
                nc.gpsimd.memset(xout[:], 0.0)
                nc.gpsimd.memset(adout[:], 0.0)
            nc.sync.dma_start(
                XO[:].rearrange("(t p) c -> p t c", p=128), xout[:])
            nc.sync.dma_start(
                AD[:].rearrange("(t p) c -> p t c", p=128), adout[:])

            # ------------- edge MLP: blocks of 8 chunks (2 groups of 4)
            escb = cpool.tile([128, NCHE, 9], F32)
            NBLK = NCHE // 8 if max_blk is None else max_blk
            if max_blk is not None:
                nc.gpsimd.memset(escb[:], 0.0)
            with (
                tc.tile_pool(name="eld", bufs=3) as eld,
                tc.tile_pool(name="ea", bufs=6) as ea,
                tc.tile_pool(name="eb", bufs=4) as eb,
                tc.tile_pool(name="ea_ps", bufs=2, space="PSUM") as eap,
                tc.tile_pool(name="et_ps", bufs=2, space="PSUM") as etp,
                tc.tile_pool(name="ee_ps", bufs=2, space="PSUM") as eep,
            ):
                et = None
                for blk in range(NBLK):
                    c0 = 8 * blk
                    if blk % 2 == 0:
                        et = eld.tile([KE, 16 * 128], BF16, tag="et")
                        nb = min(16, NCHE - c0)
                        nc.sync.dma_start(
                            et[:, : nb * 128],
                            EAT9[:, c0 * 128 : (c0 + nb) * 128])
                    eo = (blk % 2) * 8 * 128
                    y4s = []
                    q8 = eb.tile([128, 8], F32, tag="q8")
                    for h in range(2):
                        y4 = eap.tile([128, 4, EMB + KE], F32,
                                      tag=f"y4{h}")
                        for c in range(4):
                            off = eo + 128 * (4 * h + c)
                            nc.tensor.matmul(
                                y4[:, c, :],
                                lhsT=et[:, off : off + 128],
                                rhs=wec_sb[:], start=True, stop=True)
                        if stop_at >= 2:
                            u2 = ea.tile([128, 4, KE], F32, tag="u2")
                            nc.scalar.activation(u2[:], y4[:, :, EMB:],
                                                 AF.Square)
                            nc.vector.tensor_reduce(
                                q8[:, 4 * h : 4 * h + 4],
                                u2[:], axis=AX.X, op=OP.add)
                        else:
                            nc.gpsimd.memset(q8[:, 4 * h : 4 * h + 4], 1.0)
                        y4s.append(y4)
                    iv8 = eb.tile([128, 8], F32, tag="iv8")
                    if stop_at >= 3:
                        nc.scalar.activation(iv8[:], q8[:], AF.Sqrt,
                                             bias=epst[:])
                        nc.vector.reciprocal(iv8[:], iv8[:])
                    else:
                        nc.gpsimd.memset(iv8[:], 1.0)
                    for h in range(2):
                        y4 = y4s[h]
                        n4 = ea.tile([128, 4, EMB], BF16, tag=f"n4{h}")
                        if stop_at < 4:
                            nc.gpsimd.memset(n4[:], 1.0)
                        for c in range(4 if stop_at >= 4 else 0):
                            ch = 4 * h + c
                            if (c0 + ch) % 8 < 5:
                                nc.vector.tensor_scalar(
                                    n4[:, c, :], y4[:, c, :EMB],
                                    iv8[:, ch : ch + 1], 0.0,
                                    op0=OP.mult, op1=OP.max)
                            else:
                                nc.scalar.activation(
                                    n4[:, c, :], y4[:, c, :EMB], AF.Relu,
                                    scale=iv8[:, ch : ch + 1])
                        zT_ps = etp.tile([128, 2, 128], BF16, tag="zT")
                        for j in range(2 if stop_at >= 5 else 0):
                            nc.tensor.transpose(
                                out=zT_ps[:, j, :],
                                in_=n4[:, 2 * j : 2 * j + 2, :].rearrange(
                                    "p a b -> p (a b)"),
                                identity=ident[:])
                        zT = ea.tile([128, 2, 128], BF16, tag=f"zTs{h}")
                        if stop_at < 6:
                            nc.gpsimd.memset(zT[:], 0.5)
                        elif h == 0:
                            nc.vector.tensor_scalar(zT[:], zT_ps[:], 1.0,
                                                    None, op0=OP.mult)
                        else:
                            nc.scalar.activation(zT[:], zT_ps[:], AF.Copy)
                        e_ps = eep.tile([128, 4, 16], F32, tag="eps")
                        for c in range(4 if stop_at >= 6 else 0):
                            nc.tensor.matmul(
                                e_ps[:, c, :],
                                lhsT=zT[:, c // 2, :],
                                rhs=ae9_sb[:, c % 2, :],
                                start=True, stop=True)
                        if stop_at >= 7:
                            if (blk + h) % 2 == 0:
                                nc.vector.tensor_scalar(
                                    escb[:, c0 + 4 * h : c0 + 4 * h + 4, :],
                                    e_ps[:, :, 0:9], 1.0, None, op0=OP.mult)
                            else:
                                nc.scalar.activation(
                                    escb[:, c0 + 4 * h : c0 + 4 * h + 4, :],
                                    e_ps[:, :, 0:9], AF.Copy)
            nc.sync.dma_start(ESC9[:, :, :], escb[:])
    nc.compile()
    return nc


# ------------------------------------------------------------------ launch 2


def _build_launch2(KT):
    NS = KT * 128
    nc = bacc.Bacc("TRN2", target_bir_lowering=False, debug=False,
                   num_devices=NCORES)
    din = lambda n, s, d=F32: nc.dram_tensor(n, s, d, kind="ExternalInput")
    XG = din("XG", [NWIN, 128, KT, EMB], BF16)
    CMT = din("CMT", [NWIN, 24, NS], F32)
    S3H = din("S3H", [NWIN, 128, KT, 128], BF16)
    E24 = din("E24", [24, 8], F32)
    OWC = din("OWC", [128, 4, 66], BF16)    # [out_W | oa_src | oa_tgt] blocks
    WB4 = din("WB4", [128, 4, 128], BF16)   # block-diag gat_W head pairs
    HOUTS = nc.dram_tensor("HOUTS", [NPC, 66], F32, kind="ExternalOutput")

    vp = _vpaths(KT)

    with tile.TileContext(nc) as tc:
        with tc.tile_pool(name="const", bufs=1) as cpool:
            ident = cpool.tile([128, 128], BF16)
            make_identity(nc, ident[:])
            negone = cpool.tile([128, 1], F32)
            nc.gpsimd.memset(negone[:], -1.0)
            slp = cpool.tile([128, 1], F32)
            nc.gpsimd.memset(slp[:], SLOPE)
            nslp = cpool.tile([128, 1], F32)
            nc.gpsimd.memset(nslp[:], -SLOPE)
            e24_sb = cpool.tile([24, 8], F32)
            nc.sync.dma_start(e24_sb[:], E24[:])
            owc_sb = cpool.tile([128, 4, 66], BF16)
            nc.sync.dma_start(owc_sb[:], OWC[:])
            wb4_sb = cpool.tile([128, 4, 128], BF16)
            nc.sync.dma_start(wb4_sb[:], WB4[:])
            with (
                tc.tile_pool(name="w", bufs=3) as wp,
                tc.tile_pool(name="wv", bufs=6) as wv,
                tc.tile_pool(name="ws_ps", bufs=2, space="PSUM") as wsp,
                tc.tile_pool(name="wn_ps", bufs=2, space="PSUM") as wnp,
                tc.tile_pool(name="wt_ps", bufs=2, space="PSUM") as wtp,
            ):
                for w in range(NWIN):
                    xg = wp.tile([128, KT, EMB], BF16, tag="xg")
                    nc.sync.dma_start(xg[:], XG[w])
                    cmt = wp.tile([24, NS], F32, tag="cmt")
                    nc.sync.dma_start(cmt[:], CMT[w])
                    s3 = wp.tile([128, KT, 128], BF16, tag="s3")
                    nc.sync.dma_start(s3[:], S3H[w])
                    # scores: s8 = sum of components via PE
                    psu = wsp.tile([128, KT * 8 + 74], F32, tag="s8u")
                    s8_ps = psu[:, : KT * 8].rearrange(
                        "p (c i) -> p c i", i=8)
                    den_ps = psu[:, KT * 8 + 66 : KT * 8 + 74]
                    for c in range(KT):
                        nc.tensor.matmul(s8_ps[:, c, :],
                                         lhsT=cmt[:, 128 * c : 128 * (c + 1)],
                                         rhs=e24_sb[:], start=True, stop=True)
                    # w8 = exp(lrelu(s)) = max(exp(s), exp(0.01 s))
                    ex1 = wv.tile([128, KT, 8], BF16, tag="ex1")
                    nc.scalar.activation(ex1[:], s8_ps, AF.Exp)
                    ex2 = wv.tile([128, KT, 8], BF16, tag="ex2")
                    nc.scalar.activation(ex2[:], s8_ps, AF.Exp,
                                         scale=slp[:])
                    w8 = wv.tile([128, KT, 8], BF16, tag="w8")
                    nc.vector.tensor_tensor(out=w8[:], in0=ex1[:],
                                            in1=ex2[:], op=OP.max)
                    # V per chunk (DVE / Pool split) + one-hot matmuls
                    num_ps = wnp.tile([128, 512], F32, tag="num")
                    for c in range(KT):
                        V = wv.tile([128, HEADS, EMB], BF16, tag="V")
                        if vp[c] == "A":
                            w8r = wv.tile([128, HEADS, EMB], BF16,
                                          tag="w8rep")
                            nc.scalar.activation(
                                w8r[:],
                                w8[:, c, :].to_broadcast([128, 8, EMB]),
                                AF.Copy)
                            xgr = wv.tile([128, HEADS, EMB], BF16,
                                          tag="xgrep")
                            nc.scalar.activation(
                                xgr[:],
                                xg[:, c : c + 1, :].to_broadcast(
                                    [128, 8, EMB]),
                                AF.Copy)
                            nc.vector.tensor_tensor(out=V[:], in0=w8r[:],
                                                    in1=xgr[:], op=OP.mult)
                        elif vp[c] == "D":
                            nc.vector.tensor_tensor(
                                out=V[:],
                                in0=xg[:, c : c + 1, :].to_broadcast(
                                    [128, 8, EMB]),
                                in1=w8[:, c, :].to_broadcast([128, 8, EMB]),
                                op=OP.mult)
                        else:
                            nc.gpsimd.tensor_tensor(
                                out=V[:],
                                in0=xg[:, c : c + 1, :].to_broadcast(
                                    [128, 8, EMB]),
                                in1=w8[:, c, :].to_broadcast([128, 8, EMB]),
                                op=OP.mult)
                        nc.tensor.matmul(num_ps[:],
                                         lhsT=s3[:, c, :],
                                         rhs=V[:].rearrange(
                                             "p i f -> p (i f)"),
                                         start=(c == 0), stop=(c == KT - 1))
                        nc.tensor.matmul(den_ps, lhsT=s3[:, c, :],
                                         rhs=w8[:, c, :],
                                         start=(c == 0), stop=(c == KT - 1))
                    den = wv.tile([128, 8], F32, tag="dens")
                    nc.vector.tensor_scalar(den[:], den_ps, 1e-16, None,
                                            op0=OP.add)
                    nc.vector.reciprocal(den[:], den[:])
                    xh = wv.tile([128, 512], BF16, tag="xh")
                    nc.vector.tensor_tensor(
                        out=xh[:].rearrange("p (i f) -> p i f", f=EMB),
                        in0=num_ps[:].rearrange("p (i f) -> p i f", f=EMB),
                        in1=den[:].to_broadcast([128, 8, EMB]), op=OP.mult)
                    ho_ps = psu[:, KT * 8 : KT * 8 + 66]
                    # per-head W: transpose agg, W-matmul (stays f-major)
                    hh = wv.tile([128, 4, 128], BF16, tag="hh")
                    for j in range(4):
                        xT_ps = wtp.tile([128, 128], BF16, tag="xT")
                        nc.tensor.transpose(
                            out=xT_ps[:], in_=xh[:, 128 * j : 128 * (j + 1)],
                            identity=ident[:])
                        xT = wv.tile([128, 128], BF16, tag="xTs")
                        if j % 2 == 0:
                            nc.scalar.activation(xT[:], xT_ps[:], AF.Copy)
                        else:
                            nc.vector.tensor_scalar(xT[:], xT_ps[:], 1.0,
                                                    None, op0=OP.mult)
                        hT_ps = wtp.tile([128, 128], F32, tag="hT")
                        nc.tensor.matmul(hT_ps[:], lhsT=wb4_sb[:, j, :],
                                         rhs=xT[:], start=True, stop=True)
                        if j % 2 == 0:
                            nc.vector.tensor_scalar(hh[:, j, :], hT_ps[:],
                                                    1.0, None, op0=OP.mult)
                        else:
                            nc.scalar.activation(hh[:, j, :], hT_ps[:],
                                                 AF.Copy)
                    # elu(elu(.)) in f-major, batched over the 4 blocks
                    m0 = wv.tile([128, 512], BF16, tag="m0")
                    nc.vector.tensor_scalar_min(
                        m0[:], hh[:].rearrange("p a b -> p (a b)"), 0.0)
                    nc.scalar.activation(m0[:], m0[:], AF.Exp)
                    nc.scalar.activation(m0[:], m0[:], AF.Exp,
                                         bias=negone[:])
                    r0 = wv.tile([128, 512], BF16, tag="r0")
                    nc.vector.tensor_scalar(
                        r0[:], hh[:].rearrange("p a b -> p (a b)"), 0.0,
                        -1.0, op0=OP.max, op1=OP.add)
                    xh2 = wv.tile([128, 4, 128], BF16, tag="xh2")
                    nc.vector.tensor_tensor(
                        out=xh2[:].rearrange("p a b -> p (a b)"), in0=m0[:],
                        in1=r0[:], op=OP.add)
                    # out layer from f-major xh2 blocks
                    for j in range(4):
                        nc.tensor.matmul(ho_ps, lhsT=xh2[:, j, :],
                                         rhs=owc_sb[:, j, :],
                                         start=(j == 0), stop=(j == 3))
                    hrow = wv.tile([128, 66], F32, tag="hrow")
                    nc.scalar.activation(hrow[:], ho_ps, AF.Copy)
                    nc.sync.dma_start(HOUTS[128 * w : 128 * (w + 1), :],
                                      hrow[:])
    nc.compile()
    return nc


# ------------------------------------------------------------------ launch 3


def _build_launch3(KT):
    NS = KT * 128
    nc = bacc.Bacc("TRN2", target_bir_lowering=False, debug=False,
                   num_devices=NCORES)
    din = lambda n, s, d=F32: nc.dram_tensor(n, s, d, kind="ExternalInput")
    HG = din("HG", [NWIN, 128, KT, OUT], BF16)
    CM2 = din("CM2", [NWIN, 4, NS], F32)    # e9 | asrcO | atgtO | 0
    S3D = din("S3D", [NWIN, 128, NDMA3, 128], BF16)   # first NDMA3 chunks
    SRCWF = din("SRCWF", [128, NWIN, KT], F32)
    OUTT = nc.dram_tensor("OUTT", [NPC, OUT], F32, kind="ExternalOutput")

    with tile.TileContext(nc) as tc:
        with tc.tile_pool(name="const", bufs=1) as cpool:
            e4 = cpool.tile([4, 1], F32)
            nc.gpsimd.memset(e4[:], 1.0)
            slp = cpool.tile([128, 1], F32)
            nc.gpsimd.memset(slp[:], SLOPE)
            iota_bf = cpool.tile([128, 128], BF16)
            nc.gpsimd.iota(iota_bf[:], pattern=[[1, 128]], base=0,
                           channel_multiplier=0,
                           allow_small_or_imprecise_dtypes=True)
            srcwf = cpool.tile([128, NWIN, KT], F32)
            nc.sync.dma_start(srcwf[:], SRCWF[:])
            hall = cpool.tile([128, NWIN, OUT], F32)
            with (
                tc.tile_pool(name="w", bufs=3) as wp,
                tc.tile_pool(name="wv", bufs=6) as wv,
                tc.tile_pool(name="ws_ps", bufs=2, space="PSUM") as wsp,
                tc.tile_pool(name="wn_ps", bufs=2, space="PSUM") as wnp,
                tc.tile_pool(name="wd_ps", bufs=2, space="PSUM") as wdp,
            ):
                for w in range(NWIN):
                    hg = wp.tile([128, KT, OUT], BF16, tag="hg")
                    nc.sync.dma_start(hg[:], HG[w])
                    cm2 = wp.tile([4, NS], F32, tag="cm2")
                    nc.sync.dma_start(cm2[:], CM2[w])
                    s3 = wp.tile([128, KT, 128], BF16, tag="s3")
                    nc.sync.dma_start(s3[:, :NDMA3, :], S3D[w])
                    # build remaining one-hot chunks on DVE / Pool
                    for c in range(NDMA3, KT):
                        if c % 2 == 0:
                            nc.vector.tensor_scalar(
                                s3[:, c, :], iota_bf[:],
                                srcwf[:, w, c : c + 1], None,
                                op0=OP.is_equal)
                        else:
                            nc.gpsimd.tensor_scalar(
                                s3[:, c, :], iota_bf[:],
                                srcwf[:, w, c : c + 1], None,
                                op0=OP.is_equal)
                    s1_ps = wsp.tile([128, KT], F32, tag="s1")
                    for c in range(KT):
                        nc.tensor.matmul(s1_ps[:, c : c + 1],
                                         lhsT=cm2[:, 128 * c : 128 * (c + 1)],
                                         rhs=e4[:], start=True, stop=True)
                    # w1 = max(exp(s), exp(0.01 s))
                    ex1 = wv.tile([128, KT], BF16, tag="ex1")
                    nc.scalar.activation(ex1[:], s1_ps[:], AF.Exp)
                    ex2 = wv.tile([128, KT], BF16, tag="ex2")
                    nc.scalar.activation(ex2[:], s1_ps[:], AF.Exp,
                                         scale=slp[:])
                    w1 = wv.tile([128, KT], BF16, tag="w1")
                    nc.vector.tensor_tensor(out=w1[:], in0=ex1[:],
                                            in1=ex2[:], op=OP.max)
                    V1 = wv.tile([128, KT, OUT], BF16, tag="V1")
                    h3 = KT // 3
                    nc.vector.tensor_tensor(
                        out=V1[:, : 2 * h3, :], in0=hg[:, : 2 * h3, :],
                        in1=w1[:, : 2 * h3].to_broadcast(
                            [128, 2 * h3, OUT]), op=OP.mult)
                    nc.gpsimd.tensor_tensor(
                        out=V1[:, 2 * h3 :, :], in0=hg[:, 2 * h3 :, :],
                        in1=w1[:, 2 * h3 :].to_broadcast(
                            [128, KT - 2 * h3, OUT]),
                        op=OP.mult)
                    num_ps = wnp.tile([128, OUT], F32, tag="num")
                    den_ps = wdp.tile([128, 1], F32, tag="den")
                    for c in range(KT):
                        nc.tensor.matmul(num_ps[:], lhsT=s3[:, c, :],
                                         rhs=V1[:, c, :],
                                         start=(c == 0), stop=(c == KT - 1))
                        nc.tensor.matmul(den_ps[:], lhsT=s3[:, c, :],
                                         rhs=w1[:, c : c + 1],
                                         start=(c == 0), stop=(c == KT - 1))
                    den = wv.tile([128, 1], F32, tag="dens")
                    nc.vector.tensor_scalar(den[:], den_ps[:], 1e-16, None,
                                            op0=OP.add)
                    nc.vector.reciprocal(den[:], den[:])
                    h2 = wv.tile([128, OUT], F32, tag="h2")
                    nc.vector.tensor_scalar(h2[:], num_ps[:], den[:], None,
                                            op0=OP.mult)
                    m0 = wv.tile([128, OUT], F32, tag="m0")
                    nc.vector.tensor_scalar_min(m0[:], h2[:], 0.0)
                    nc.scalar.activation(m0[:], m0[:], AF.Exp)
                    r0 = wv.tile([128, OUT], F32, tag="r0")
                    nc.vector.tensor_scalar(r0[:], h2[:], 0.0, -1.0,
                                            op0=OP.max, op1=OP.add)
                    nc.vector.tensor_tensor(out=hall[:, w, :], in0=m0[:],
                                            in1=r0[:], op=OP.add)
            with tc.tile_pool(name="fin", bufs=1) as fin:
                ex = fin.tile([128, NWIN, OUT], F32)
                nc.scalar.activation(ex[:], hall[:], AF.Exp)
                sm = fin.tile([128, NWIN], F32)
                nc.vector.tensor_reduce(sm[:], ex[:], axis=AX.X, op=OP.add)
                nc.scalar.activation(sm[:], sm[:], AF.Ln)
                res = fin.tile([128, NWIN, OUT], F32)
                nc.vector.tensor_tensor(
                    out=res[:], in0=hall[:],
                    in1=sm[:].to_broadcast([128, NWIN, OUT]), op=OP.subtract)
                nc.sync.dma_start(
                    OUTT[:].rearrange("(w p) f -> p w f", p=128), res[:])
    nc.compile()
    return nc


# ------------------------------------------------------------------ driver


def kernel(X, edge_attr, w_node, b_node, g_node, beta_node,
           w_edge, b_edge, g_edge, beta_edge,
           gat_W, gat_a, out_W, out_a,
           edge_index, matched_car_infra_nodes):
    import ml_dtypes
    import time as _time

    bf = lambda a: np.ascontiguousarray(np.asarray(a, np.float32)).astype(
        ml_dtypes.bfloat16)
    f32 = lambda a: np.ascontiguousarray(np.asarray(a, np.float32))

    X = f32(X)
    ea = f32(edge_attr)
    w_node = f32(w_node); b_node = f32(b_node); g_node = f32(g_node)
    beta_node = f32(beta_node)
    w_edge = f32(w_edge); b_edge = f32(b_edge); g_edge = f32(g_edge)
    beta_edge = f32(beta_edge)
    gW = f32(gat_W); ga = f32(gat_a); oW = f32(out_W); oa = f32(out_a)
    assert np.abs(beta_node).max() < 1e-6 and np.abs(beta_edge).max() < 1e-6

    per_core, pnode, origin, KT, srcw_of, ptgt = _prep(edge_index)
    NS = KT * 128
    NCHE = NWIN * KT
    NCHE4 = ((NCHE + 15) // 16) * 16

    # ---- LN-folded weights (centered + Cholesky u-columns)
    def fold(Wb, bb, g, kdim):
        Wfull = np.concatenate([Wb, bb[None, :]], 0)          # [k, 64]
        m = Wfull.mean(axis=1)                                 # [k]
        Wc = Wfull - m[:, None]
        M = Wc @ Wc.T + 1e-10 * np.eye(kdim)
        B = np.linalg.cholesky(M) / np.sqrt(EMB)
        return np.concatenate([Wc * g[None, :], B], 1)         # [k, 64+k]

    WNC = bf(fold(w_node, b_node, g_node, NODE_DIM + 1))
    WEC = bf(fold(w_edge, b_edge, g_edge, EA_DIM + 1))
    WAB = np.zeros((EMB, 16), np.float32)
    for i in range(HEADS):
        WAB[:, i] = gW[i] @ ga[i, :OUT]
        WAB[:, 8 + i] = gW[i] @ ga[i, OUT : 2 * OUT]
    WAB = bf(WAB)
    AE9 = np.zeros((EMB, 16), np.float32)
    for i in range(HEADS):
        AE9[:, i] = ga[i, 2 * OUT :]
    AE9[:, 8] = oa[2 * OUT :]
    A2 = np.zeros((2, 128, 16), np.float32)
    A2[0, :EMB] = AE9
    A2[1, EMB:] = AE9
    AE9 = bf(A2.transpose(1, 0, 2))

    # ---- launch 1 inputs
    Xp = np.zeros((NPN, NODE_DIM + 1), np.float32)
    valid = origin >= 0
    Xp[valid, :NODE_DIM] = X[origin[valid]]
    Xp[:, NODE_DIM] = 1.0
    src = np.asarray(edge_index[0]).astype(np.int64)

    in_maps1 = []
    for k in range(NCORES):
        eslot = per_core[k]                                    # [NWIN, NS]
        eat = np.zeros((NCHE4 * 128, EA_DIM + 1), np.float32)
        es = eslot.reshape(-1)
        m = es >= 0
        eat[: NS * NWIN][m, :EA_DIM] = ea[es[m]]
        eat[: NS * NWIN][m, EA_DIM] = 1.0
        in_maps1.append(dict(
            XT17=bf(Xp[k * NPC : (k + 1) * NPC].T),
            WNC=WNC, WAB=WAB, AE9=AE9,
            EAT9=bf(eat.T), WEC=WEC))

    nc1 = _build_launch1(NCHE4)
    kernel.nc1 = nc1
    _t = _time.perf_counter()
    res1 = run_bass_kernel_spmd(nc1, in_maps1, core_ids=list(range(NCORES)))
    kernel.wall1 = _time.perf_counter() - _t

    # ---- host: assemble tables, gather per-slot inputs for launch 2
    XF = np.zeros((NPN, EMB), ml_dtypes.bfloat16)
    ADF = np.zeros((NPN, 16), np.float32)
    ESCF = []
    for k in range(NCORES):
        XF[k * NPC : (k + 1) * NPC] = res1.results[k]["XO"]
        ADF[k * NPC : (k + 1) * NPC] = res1.results[k]["AD"]
        # ESC9 [128, NCHE4, 9] -> slot-major [NWIN, NS, 9]
        e9 = np.asarray(res1.results[k]["ESC9"], np.float32)[:, :NCHE, :]
        e9 = e9.transpose(1, 0, 2).reshape(NWIN, NS, 9)
        ESCF.append(e9)

    # one-hot S3 per core (shared by launches 2 and 3)
    in_maps2 = []
    s3_cores = []
    for k in range(NCORES):
        eslot = per_core[k]
        es = eslot.reshape(NWIN, NS)
        m = es >= 0
        tgtrow = np.zeros((NWIN, NS), np.int64)
        tgtrow[m] = ptgt[es[m]]
        srcw = np.full((NWIN, NS), -1, np.int64)
        srcw[m] = srcw_of[es[m]]

        XGk = np.zeros((NWIN, NS, EMB), ml_dtypes.bfloat16)
        XGk[m] = XF[tgtrow[m]]
        CMTk = np.zeros((NWIN, 24, NS), np.float32)
        CMTk[:, 0:8, :] = ESCF[k][:, :, 0:8].transpose(0, 2, 1)
        srcrow_k = np.zeros((NWIN, NS), np.int64)
        # src row = core base + win*128 + srcw
        wid = np.arange(NWIN)[:, None]
        srcrow_k[m] = (k * NPC + (wid + np.zeros_like(srcw))[m] * 128
                       + srcw[m])
        asrc = np.zeros((NWIN, NS, 8), np.float32)
        asrc[m] = ADF[srcrow_k[m], 0:8]
        atgt = np.zeros((NWIN, NS, 8), np.float32)
        atgt[m] = ADF[tgtrow[m], 8:16]
        CMTk[:, 8:16, :] = asrc.transpose(0, 2, 1)
        CMTk[:, 16:24, :] = atgt.transpose(0, 2, 1)

        S3k = np.zeros((NWIN, NS, 128), ml_dtypes.bfloat16)
        ww, ss = np.nonzero(m)
        S3k[ww, ss, srcw[ww, ss]] = 1.0
        S3k = S3k.reshape(NWIN, KT, 128, 128).transpose(0, 2, 1, 3)
        s3_cores.append(np.ascontiguousarray(S3k))

        WB4 = np.zeros((128, 4, 128), np.float32)
        for j in range(4):
            for il in range(2):
                WB4[64 * il : 64 * il + 64, j,
                    64 * il : 64 * il + 64] = gW[2 * j + il]
        E24 = np.zeros((24, 8), np.float32)
        for i in range(8):
            E24[i, i] = 1.0
            E24[8 + i, i] = 1.0
            E24[16 + i, i] = 1.0
        OWC = np.zeros((512, 66), np.float32)
        OWC[:, 0:64] = oW
        OWC[:, 64] = oW @ oa[:OUT]
        OWC[:, 65] = oW @ oa[OUT : 2 * OUT]
        in_maps2.append(dict(
            XG=_slotmaj(XGk, KT, EMB),
            CMT=CMTk,
            S3H=s3_cores[k],
            E24=E24,
            OWC=bf(np.ascontiguousarray(
                OWC.reshape(4, 128, 66).transpose(1, 0, 2))),
            WB4=bf(WB4),
        ))

    nc2 = _build_launch2(KT)
    kernel.nc2 = nc2
    _t = _time.perf_counter()
    res2 = run_bass_kernel_spmd(nc2, in_maps2, core_ids=list(range(NCORES)))
    kernel.wall2 = _time.perf_counter() - _t

    # ---- host: assemble h_out table, gather for launch 3
    HF = np.zeros((NPN, 66), np.float32)
    for k in range(NCORES):
        HF[k * NPC : (k + 1) * NPC] = res2.results[k]["HOUTS"]
    HFb = HF[:, 0:64].astype(ml_dtypes.bfloat16)

    in_maps3 = []
    for k in range(NCORES):
        eslot = per_core[k]
        es = eslot.reshape(NWIN, NS)
        m = es >= 0
        tgtrow = np.zeros((NWIN, NS), np.int64)
        tgtrow[m] = ptgt[es[m]]
        srcw = np.full((NWIN, NS), -1, np.int64)
        srcw[m] = srcw_of[es[m]]
        wid = np.arange(NWIN)[:, None]
        srcrow_k = np.zeros((NWIN, NS), np.int64)
        srcrow_k[m] = (k * NPC + (wid + np.zeros_like(srcw))[m] * 128
                       + srcw[m])

        HGk = np.zeros((NWIN, NS, OUT), ml_dtypes.bfloat16)
        HGk[m] = HFb[tgtrow[m]]
        CM2k = np.zeros((NWIN, 4, NS), np.float32)
        CM2k[:, 0, :] = ESCF[k][:, :, 8]
        a_s = np.zeros((NWIN, NS), np.float32)
        a_s[m] = HF[srcrow_k[m], 64]
        a_t = np.zeros((NWIN, NS), np.float32)
        a_t[m] = HF[tgtrow[m], 65]
        CM2k[:, 1, :] = a_s
        CM2k[:, 2, :] = a_t
        srcwf_f = srcw.reshape(NWIN, KT, 128).transpose(2, 0, 1).astype(
            np.float32)
        in_maps3.append(dict(
            HG=_slotmaj(HGk, KT, OUT),
            CM2=CM2k,
            S3D=np.ascontiguousarray(s3_cores[k][:, :, :NDMA3, :]),
            SRCWF=np.ascontiguousarray(srcwf_f)))

    nc3 = _build_launch3(KT)
    kernel.nc3 = nc3
    _t = _time.perf_counter()
    res3 = run_bass_kernel_spmd(nc3, in_maps3, core_ids=list(range(NCORES)))
    kernel.wall3 = _time.perf_counter() - _t

    outp = np.zeros((NPN, OUT), np.float32)
    for k in range(NCORES):
        outp[k * NPC : (k + 1) * NPC] = res3.results[k]["OUTT"]
    out = np.zeros((N, OUT), np.float32)
    valid = origin >= 0
    out[origin[valid]] = outp[valid]
    return out


def _slotmaj(A, KT, F):
    """[NWIN, NS, F] with slot s=(c*128+p) -> [NWIN, 128, KT, F]."""
    NW = A.shape[0]
    return np.ascontiguousarray(
        A.reshape(NW, KT, 128, F).transpose(0, 2, 1, 3))


# revision 51
# speedup vs baseline: 1.0090x; 1.0090x over previous
"""AA_GAT on 8 trn2 cores (self-contained), v2.

Three launches; host does layout/gather only between launches.

L1: node MLP (nodes sharded 1/8 per core) + edge MLP (edges sharded by
    src-window). LN via Cholesky trick: y' = centered pre-LN output and
    u-columns come out of one matmul; var = sum(u^2)/64. beta=0 lets
    relu commute with the 1/sigma scale, so the only PSUM->SBUF bridge
    is a plain Relu; the iv scale is applied to the tiny outputs
    (adots 16 cols, esc 9 cols, x 64 cols once per node tile).
L2: layer-1 8-head edge pass per src window. Scores summed on PE from
    a host-transposed component table (esc8|asrc8|atgt8); exp on Act;
    per-edge value weighting V = w8 (x) xg via three engine paths
    (Act-replicate + DVE-stt / DVE tensor_tensor / Pool stt), one-hot
    segment-sum matmuls (host-prebuilt S3), elu(elu(.)), out-layer
    h_out = xh @ out_W + alpha dots.
L3: out-layer edge pass (same slot layout), then batched log_softmax.
"""

import numpy as np

import concourse.bass as bass
import concourse.mybir as mybir
import concourse.tile as tile
from concourse import bacc
from concourse.bass_utils import run_bass_kernel_spmd
from concourse.masks import make_identity

F32 = mybir.dt.float32
BF16 = mybir.dt.bfloat16
AF = mybir.ActivationFunctionType
OP = mybir.AluOpType
AX = mybir.AxisListType

N = 50000
E = 1_000_000
NODE_DIM = 16
EMB = 64
OUT = 64
HEADS = 8
EA_DIM = 8
SLOPE = 0.01
LN_EPS = 1e-5

NCORES = 8
NWIN = 49                 # windows (128 src nodes) per core
NPC = NWIN * 128          # 6272 nodes per core
NPN = NCORES * NPC        # 50176 padded node count
NWTOT = NCORES * NWIN     # 392 windows total
NDMA3 = 10                # launch-3 one-hot chunks loaded via DMA

# L2 per-chunk V-path assignment (tuned): 'B' Act-replicate + DVE stt,
# 'A' DVE tensor_tensor broadcast, 'C' Pool stt broadcast.


def _vpaths(KT):
    # A = Act double-replicate + packed DVE TT (2x), D = DVE TT broadcast,
    # P = Pool TT broadcast; 2:11:7 split
    order = "DPDPADDPDPDPDPADDPDP" * 4
    return [order[c % len(order)] for c in range(KT)]


# ------------------------------------------------------------------ host prep


def _prep(edge_index):
    """Degree-balanced node->window permutation and edge slot layout."""
    src = np.asarray(edge_index[0]).astype(np.int64)
    tgt = np.asarray(edge_index[1]).astype(np.int64)

    deg = np.bincount(src, minlength=N).astype(np.int64)
    # greedy: big-degree nodes first, into least-loaded window with space
    order = np.argsort(-deg, kind="stable")
    wload = np.zeros(NWTOT, np.int64)
    wcnt = np.zeros(NWTOT, np.int64)
    wnodes = [[] for _ in range(NWTOT)]
    import heapq

    heap = [(0, 0, w) for w in range(NWTOT)]
    heapq.heapify(heap)
    for n in order:
        while True:
            load, cnt, w = heapq.heappop(heap)
            if wcnt[w] < 128:
                break
        wnodes[w].append(n)
        wload[w] += deg[n]
        wcnt[w] += 1
        if wcnt[w] < 128:
            heapq.heappush(heap, (wload[w], wcnt[w], w))
    # order windows by load, snake-assign to cores for balance
    worder = np.argsort(-wload, kind="stable")
    core_wins = [[] for _ in range(NCORES)]
    fwd = True
    i = 0
    while i < NWTOT:
        rng = range(NCORES) if fwd else range(NCORES - 1, -1, -1)
        for k in rng:
            if i < NWTOT:
                core_wins[k].append(worder[i])
                i += 1
        fwd = not fwd
    # global permuted row id: core k, local window j, slot s
    pnode = np.full(N, -1, np.int64)
    origin = np.full(NPN, -1, np.int64)
    for k in range(NCORES):
        for j, w in enumerate(core_wins[k]):
            base = k * NPC + j * 128
            nodes = wnodes[w]
            for s, n in enumerate(nodes):
                pnode[n] = base + s
                origin[base + s] = n
    assert (pnode >= 0).all()

    psrc = pnode[src]
    ptgt = pnode[tgt]
    core_of = psrc // NPC
    win_of = (psrc % NPC) // 128
    srcw_of = psrc % 128

    KT = 0
    buckets = {}
    for k in range(NCORES):
        mk = core_of == k
        idx_k = np.nonzero(mk)[0]
        w = win_of[idx_k]
        for ww in range(NWIN):
            el = idx_k[w == ww]
            buckets[(k, ww)] = el
            KT = max(KT, (len(el) + 127) // 128)
    NS = KT * 128          # slots per window

    per_core = []
    for k in range(NCORES):
        eslot = np.full((NWIN, NS), -1, np.int64)     # edge id per slot
        for ww in range(NWIN):
            el = buckets[(k, ww)]
            eslot[ww, : len(el)] = el
        per_core.append(eslot)
    return per_core, pnode, origin, KT, srcw_of, ptgt


# ------------------------------------------------------------------ launch 1


def _build_launch1(NCHE, skip_node=False, max_blk=None, stop_at=99):
    """Node MLP (49 tiles) + edge MLP (NCHE chunks)."""
    nc = bacc.Bacc("TRN2", target_bir_lowering=False, debug=False,
                   num_devices=NCORES)
    din = lambda n, s, d=F32: nc.dram_tensor(n, s, d, kind="ExternalInput")
    XT17 = din("XT17", [NODE_DIM + 1, NPC], BF16)
    WNC = din("WNC", [NODE_DIM + 1, EMB + NODE_DIM + 1], BF16)
    WAB = din("WAB", [EMB, 16], BF16)
    AE9 = din("AE9", [128, 2, 16], BF16)     # [AE9;0] and [0;AE9] halves
    EAT9 = din("EAT9", [EA_DIM + 1, NCHE * 128], BF16)
    WEC = din("WEC", [EA_DIM + 1, EMB + EA_DIM + 1], BF16)

    XO = nc.dram_tensor("XO", [NPC, EMB], BF16, kind="ExternalOutput")
    AD = nc.dram_tensor("AD", [NPC, 16], F32, kind="ExternalOutput")
    ESC9 = nc.dram_tensor("ESC9", [128, NCHE, 9], F32, kind="ExternalOutput")

    KN = NODE_DIM + 1   # 17 u-cols (node)
    KE = EA_DIM + 1     # 9 u-cols (edge)

    with tile.TileContext(nc) as tc:
        with tc.tile_pool(name="const", bufs=1) as cpool:
            ident = cpool.tile([128, 128], BF16)
            make_identity(nc, ident[:])
            epst = cpool.tile([128, 1], F32)
            nc.gpsimd.memset(epst[:], LN_EPS)
            wnc_sb = cpool.tile([KN, EMB + KN], BF16)
            nc.sync.dma_start(wnc_sb[:], WNC[:])
            wab_sb = cpool.tile([EMB, 16], BF16)
            nc.sync.dma_start(wab_sb[:], WAB[:])
            ae9_sb = cpool.tile([128, 2, 16], BF16)
            nc.sync.dma_start(ae9_sb[:], AE9[:])
            wec_sb = cpool.tile([KE, EMB + KE], BF16)
            nc.sync.dma_start(wec_sb[:], WEC[:])

            # ------------- node MLP: 49 tiles, batch 4 for stats
            xout = cpool.tile([128, NWIN, EMB], BF16)
            adout = cpool.tile([128, NWIN, 16], F32)
            xt17 = cpool.tile([KN, NPC], BF16)
            nc.sync.dma_start(xt17[:], XT17[:])
            NG = 0 if skip_node else (NWIN + 3) // 4
            with (
                tc.tile_pool(name="na", bufs=5) as na,
                tc.tile_pool(name="na_ps", bufs=3, space="PSUM") as nap,
                tc.tile_pool(name="nt_ps", bufs=2, space="PSUM") as ntp,
                tc.tile_pool(name="nad_ps", bufs=2, space="PSUM") as nadp,
            ):
                for g in range(NG):
                    t0 = 4 * g
                    nt = min(4, NWIN - t0)
                    y4 = nap.tile([128, 4, EMB + KN], F32, tag="y4")
                    for t in range(nt):
                        nc.tensor.matmul(
                            y4[:, t, :],
                            lhsT=xt17[:, 128 * (t0 + t) : 128 * (t0 + t + 1)],
                            rhs=wnc_sb[:], start=True, stop=True)
                    u2 = na.tile([128, 4, KN], F32, tag="u2")
                    nc.scalar.activation(u2[:, :nt, :], y4[:, :nt, EMB:],
                                         AF.Square)
                    q = na.tile([128, 4], F32, tag="q")
                    nc.vector.tensor_reduce(q[:, :nt], u2[:, :nt, :],
                                            axis=AX.X, op=OP.add)
                    iv = na.tile([128, 4], F32, tag="iv")
                    nc.scalar.activation(iv[:, :nt], q[:, :nt], AF.Sqrt,
                                         bias=epst[:])
                    nc.vector.reciprocal(iv[:, :nt], iv[:, :nt])
                    for t in range(nt):
                        # x = max(iv*y'g, 0) directly into the table row
                        nc.vector.tensor_scalar(
                            xout[:, t0 + t, :], y4[:, t, :EMB],
                            iv[:, t : t + 1], 0.0, op0=OP.mult, op1=OP.max)
                        rT_ps = ntp.tile([EMB, 128], BF16, tag="rT")
                        nc.tensor.transpose(out=rT_ps[:],
                                            in_=xout[:, t0 + t, :],
                                            identity=ident[:])
                        rT = na.tile([EMB, 128], BF16, tag="rTs")
                        nc.scalar.activation(rT[:], rT_ps[:], AF.Copy)
                        a_ps = nadp.tile([128, 16], F32, tag="aps")
                        nc.tensor.matmul(a_ps[:], lhsT=rT[:], rhs=wab_sb[:],
                                         start=True, stop=True)
                        nc.vector.tensor_scalar(
                            adout[:, t0 + t, :], a_ps[:], 1.0, None,
                            op0=OP.mult)
            if skip_node:
                nc.gpsimd.memset(xout[:], 0.0)
                nc.gpsimd.memset(adout[:], 0.0)
            nc.sync.dma_start(
                XO[:].rearrange("(t p) c -> p t c", p=128), xout[:])
            nc.sync.dma_start(
                AD[:].rearrange("(t p) c -> p t c", p=128), adout[:])

            # ------------- edge MLP: blocks of 8 chunks (2 groups of 4)
            escb = cpool.tile([128, NCHE, 9], F32)
            NBLK = NCHE // 8 if max_blk is None else max_blk
            if max_blk is not None:
                nc.gpsimd.memset(escb[:], 0.0)
            with (
                tc.tile_pool(name="eld", bufs=3) as eld,
                tc.tile_pool(name="ea", bufs=6) as ea,
                tc.tile_pool(name="eb", bufs=4) as eb,
                tc.tile_pool(name="ea_ps", bufs=2, space="PSUM") as eap,
                tc.tile_pool(name="et_ps", bufs=2, space="PSUM") as etp,
                tc.tile_pool(name="ee_ps", bufs=2, space="PSUM") as eep,
            ):
                et = None
                for blk in range(NBLK):
                    c0 = 8 * blk
                    if blk % 2 == 0:
                        et = eld.tile([KE, 16 * 128], BF16, tag="et")
                        nb = min(16, NCHE - c0)
                        nc.sync.dma_start(
                            et[:, : nb * 128],
                            EAT9[:, c0 * 128 : (c0 + nb) * 128])
                    eo = (blk % 2) * 8 * 128
                    y4s = []
                    q8 = eb.tile([128, 8], F32, tag="q8")
                    for h in range(2):
                        y4 = eap.tile([128, 4, EMB + KE], F32,
                                      tag=f"y4{h}")
                        for c in range(4):
                            off = eo + 128 * (4 * h + c)
                            nc.tensor.matmul(
                                y4[:, c, :],
                                lhsT=et[:, off : off + 128],
                                rhs=wec_sb[:], start=True, stop=True)
                        if stop_at >= 2:
                            u2 = ea.tile([128, 4, KE], F32, tag="u2")
                            nc.scalar.activation(u2[:], y4[:, :, EMB:],
                                                 AF.Square)
                            nc.vector.tensor_reduce(
                                q8[:, 4 * h : 4 * h + 4],
                                u2[:], axis=AX.X, op=OP.add)
                        else:
                            nc.gpsimd.memset(q8[:, 4 * h : 4 * h + 4], 1.0)
                        y4s.append(y4)
                    iv8 = eb.tile([128, 8], F32, tag="iv8")
                    if stop_at >= 3:
                        nc.scalar.activation(iv8[:], q8[:], AF.Sqrt,
                                             bias=epst[:])
                        nc.vector.reciprocal(iv8[:], iv8[:])
                    else:
                        nc.gpsimd.memset(iv8[:], 1.0)
                    for h in range(2):
                        y4 = y4s[h]
                        n4 = ea.tile([128, 4, EMB], BF16, tag=f"n4{h}")
                        if stop_at < 4:
                            nc.gpsimd.memset(n4[:], 1.0)
                        for c in range(4 if stop_at >= 4 else 0):
                            ch = 4 * h + c
                            if (c0 + ch) % 8 < 5:
                                nc.vector.tensor_scalar(
                                    n4[:, c, :], y4[:, c, :EMB],
                                    iv8[:, ch : ch + 1], 0.0,
                                    op0=OP.mult, op1=OP.max)
                            else:
                                nc.scalar.activation(
                                    n4[:, c, :], y4[:, c, :EMB], AF.Relu,
                                    scale=iv8[:, ch : ch + 1])
                        zT_ps = etp.tile([128, 2, 128], BF16, tag="zT")
                        for j in range(2 if stop_at >= 5 else 0):
                            nc.tensor.transpose(
                                out=zT_ps[:, j, :],
                                in_=n4[:, 2 * j : 2 * j + 2, :].rearrange(
                                    "p a b -> p (a b)"),
                                identity=ident[:])
                        zT = ea.tile([128, 2, 128], BF16, tag=f"zTs{h}")
                        if stop_at < 6:
                            nc.gpsimd.memset(zT[:], 0.5)
                        elif h == 0:
                            nc.vector.tensor_scalar(zT[:], zT_ps[:], 1.0,
                                                    None, op0=OP.mult)
                        else:
                            nc.scalar.activation(zT[:], zT_ps[:], AF.Copy)
                        e_ps = eep.tile([128, 4, 16], F32, tag="eps")
                        for c in range(4 if stop_at >= 6 else 0):
                            nc.tensor.matmul(
                                e_ps[:, c, :],
                                lhsT=zT[:, c // 2, :],
                                rhs=ae9_sb[:, c % 2, :],
                                start=True, stop=True)
                        if stop_at >= 7:
                            if (blk + h) % 2 == 0:
                                nc.vector.tensor_scalar(
                                    escb[:, c0 + 4 * h : c0 + 4 * h + 4, :],
                                    e_ps[:, :, 0:9], 1.0, None, op0=OP.mult)
                            else:
                                nc.scalar.activation(
                                    escb[:, c0 + 4 * h : c0 + 4 * h + 4, :],
                                    e_ps[:, :, 0:9], AF.Copy)
            nc.sync.dma_start(ESC9[:, :, :], escb[:])
    nc.compile()
    return nc


# ------------------------------------------------------------------ launch 2


def _build_launch2(KT):
    NS = KT * 128
    nc = bacc.Bacc("TRN2", target_bir_lowering=False, debug=False,
                   num_devices=NCORES)
    din = lambda n, s, d=F32: nc.dram_tensor(n, s, d, kind="ExternalInput")
    XG = din("XG", [NWIN, 128, KT, EMB], BF16)
    CMT = din("CMT", [NWIN, 24, NS], F32)
    S3H = din("S3H", [NWIN, 128, KT, 128], BF16)
    E24 = din("E24", [24, 8], F32)
    OWC = din("OWC", [128, 4, 66], BF16)    # [out_W | oa_src | oa_tgt] blocks
    WB4 = din("WB4", [128, 4, 128], BF16)   # block-diag gat_W head pairs
    HOUTS = nc.dram_tensor("HOUTS", [NPC, 66], F32, kind="ExternalOutput")

    vp = _vpaths(KT)

    with tile.TileContext(nc) as tc:
        with tc.tile_pool(name="const", bufs=1) as cpool:
            ident = cpool.tile([128, 128], BF16)
            make_identity(nc, ident[:])
            negone = cpool.tile([128, 1], F32)
            nc.gpsimd.memset(negone[:], -1.0)
            slp = cpool.tile([128, 1], F32)
            nc.gpsimd.memset(slp[:], SLOPE)
            nslp = cpool.tile([128, 1], F32)
            nc.gpsimd.memset(nslp[:], -SLOPE)
            e24_sb = cpool.tile([24, 8], F32)
            nc.sync.dma_start(e24_sb[:], E24[:])
            owc_sb = cpool.tile([128, 4, 66], BF16)
            nc.sync.dma_start(owc_sb[:], OWC[:])
            wb4_sb = cpool.tile([128, 4, 128], BF16)
            nc.sync.dma_start(wb4_sb[:], WB4[:])
            with (
                tc.tile_pool(name="w", bufs=3) as wp,
                tc.tile_pool(name="wv", bufs=6) as wv,
                tc.tile_pool(name="ws_ps", bufs=2, space="PSUM") as wsp,
                tc.tile_pool(name="wn_ps", bufs=2, space="PSUM") as wnp,
                tc.tile_pool(name="wt_ps", bufs=2, space="PSUM") as wtp,
            ):
                for w in range(NWIN):
                    xg = wp.tile([128, KT, EMB], BF16, tag="xg")
                    nc.sync.dma_start(xg[:], XG[w])
                    cmt = wp.tile([24, NS], F32, tag="cmt")
                    nc.sync.dma_start(cmt[:], CMT[w])
                    s3 = wp.tile([128, KT, 128], BF16, tag="s3")
                    nc.sync.dma_start(s3[:], S3H[w])
                    # scores: s8 = sum of components via PE
                    psu = wsp.tile([128, KT * 8 + 74], F32, tag="s8u")
                    s8_ps = psu[:, : KT * 8].rearrange(
                        "p (c i) -> p c i", i=8)
                    den_ps = psu[:, KT * 8 + 66 : KT * 8 + 74]
                    for c in range(KT):
                        nc.tensor.matmul(s8_ps[:, c, :],
                                         lhsT=cmt[:, 128 * c : 128 * (c + 1)],
                                         rhs=e24_sb[:], start=True, stop=True)
                    # w8 = exp(lrelu(s)) = max(exp(s), exp(0.01 s))
                    ex1 = wv.tile([128, KT, 8], BF16, tag="ex1")
                    nc.scalar.activation(ex1[:], s8_ps, AF.Exp)
                    ex2 = wv.tile([128, KT, 8], BF16, tag="ex2")
                    nc.scalar.activation(ex2[:], s8_ps, AF.Exp,
                                         scale=slp[:])
                    w8 = wv.tile([128, KT, 8], BF16, tag="w8")
                    nc.vector.tensor_tensor(out=w8[:], in0=ex1[:],
                                            in1=ex2[:], op=OP.max)
                    # V per chunk (DVE / Pool split) + one-hot matmuls
                    num_ps = wnp.tile([128, 512], F32, tag="num")
                    for c in range(KT):
                        V = wv.tile([128, HEADS, EMB], BF16, tag="V")
                        if vp[c] == "A":
                            w8r = wv.tile([128, HEADS, EMB], BF16,
                                          tag="w8rep")
                            nc.scalar.activation(
                                w8r[:],
                                w8[:, c, :].to_broadcast([128, 8, EMB]),
                                AF.Copy)
                            xgr = wv.tile([128, HEADS, EMB], BF16,
                                          tag="xgrep")
                            nc.scalar.activation(
                                xgr[:],
                                xg[:, c : c + 1, :].to_broadcast(
                                    [128, 8, EMB]),
                                AF.Copy)
                            nc.vector.tensor_tensor(out=V[:], in0=w8r[:],
                                                    in1=xgr[:], op=OP.mult)
                        elif vp[c] == "D":
                            nc.vector.tensor_tensor(
                                out=V[:],
                                in0=xg[:, c : c + 1, :].to_broadcast(
                                    [128, 8, EMB]),
                                in1=w8[:, c, :].to_broadcast([128, 8, EMB]),
                                op=OP.mult)
                        else:
                            nc.gpsimd.tensor_tensor(
                                out=V[:],
                                in0=xg[:, c : c + 1, :].to_broadcast(
                                    [128, 8, EMB]),
                                in1=w8[:, c, :].to_broadcast([128, 8, EMB]),
                                op=OP.mult)
                        nc.tensor.matmul(num_ps[:],
                                         lhsT=s3[:, c, :],
                                         rhs=V[:].rearrange(
                                             "p i f -> p (i f)"),
                                         start=(c == 0), stop=(c == KT - 1))
                        nc.tensor.matmul(den_ps, lhsT=s3[:, c, :],
                                         rhs=w8[:, c, :],
                                         start=(c == 0), stop=(c == KT - 1))
                    den = wv.tile([128, 8], F32, tag="dens")
                    nc.vector.tensor_scalar(den[:], den_ps, 1e-16, None,
                                            op0=OP.add)
                    nc.vector.reciprocal(den[:], den[:])
                    xh = wv.tile([128, 512], BF16, tag="xh")
                    nc.vector.tensor_tensor(
                        out=xh[:].rearrange("p (i f) -> p i f", f=EMB),
                        in0=num_ps[:].rearrange("p (i f) -> p i f", f=EMB),
                        in1=den[:].to_broadcast([128, 8, EMB]), op=OP.mult)
                    ho_ps = psu[:, KT * 8 : KT * 8 + 66]
                    # per-head W: transpose agg, W-matmul (stays f-major)
                    hh = wv.tile([128, 4, 128], BF16, tag="hh")
                    for j in range(4):
                        xT_ps = wtp.tile([128, 128], BF16, tag="xT")
                        nc.tensor.transpose(
                            out=xT_ps[:], in_=xh[:, 128 * j : 128 * (j + 1)],
                            identity=ident[:])
                        xT = wv.tile([128, 128], BF16, tag="xTs")
                        if j % 2 == 0:
                            nc.scalar.activation(xT[:], xT_ps[:], AF.Copy)
                        else:
                            nc.vector.tensor_scalar(xT[:], xT_ps[:], 1.0,
                                                    None, op0=OP.mult)
                        hT_ps = wtp.tile([128, 128], F32, tag="hT")
                        nc.tensor.matmul(hT_ps[:], lhsT=wb4_sb[:, j, :],
                                         rhs=xT[:], start=True, stop=True)
                        if j % 2 == 0:
                            nc.vector.tensor_scalar(hh[:, j, :], hT_ps[:],
                                                    1.0, None, op0=OP.mult)
                        else:
                            nc.scalar.activation(hh[:, j, :], hT_ps[:],
                                                 AF.Copy)
                    # elu(elu(.)) in f-major, batched over the 4 blocks
                    m0 = wv.tile([128, 512], BF16, tag="m0")
                    nc.vector.tensor_scalar_min(
                        m0[:], hh[:].rearrange("p a b -> p (a b)"), 0.0)
                    nc.scalar.activation(m0[:], m0[:], AF.Exp)
                    nc.scalar.activation(m0[:], m0[:], AF.Exp,
                                         bias=negone[:])
                    r0 = wv.tile([128, 512], BF16, tag="r0")
                    nc.vector.tensor_scalar(
                        r0[:], hh[:].rearrange("p a b -> p (a b)"), 0.0,
                        -1.0, op0=OP.max, op1=OP.add)
                    xh2 = wv.tile([128, 4, 128], BF16, tag="xh2")
                    nc.vector.tensor_tensor(
                        out=xh2[:].rearrange("p a b -> p (a b)"), in0=m0[:],
                        in1=r0[:], op=OP.add)
                    # out layer from f-major xh2 blocks
                    for j in range(4):
                        nc.tensor.matmul(ho_ps, lhsT=xh2[:, j, :],
                                         rhs=owc_sb[:, j, :],
                                         start=(j == 0), stop=(j == 3))
                    hrow = wv.tile([128, 66], F32, tag="hrow")
                    nc.scalar.activation(hrow[:], ho_ps, AF.Copy)
                    nc.sync.dma_start(HOUTS[128 * w : 128 * (w + 1), :],
                                      hrow[:])
    nc.compile()
    return nc


# ------------------------------------------------------------------ launch 3


def _build_launch3(KT):
    NS = KT * 128
    nc = bacc.Bacc("TRN2", target_bir_lowering=False, debug=False,
                   num_devices=NCORES)
    din = lambda n, s, d=F32: nc.dram_tensor(n, s, d, kind="ExternalInput")
    HG = din("HG", [NWIN, 128, KT, OUT], BF16)
    CM2 = din("CM2", [NWIN, 4, NS], F32)    # e9 | asrcO | atgtO | 0
    S3D = din("S3D", [NWIN, 128, NDMA3, 128], BF16)   # first NDMA3 chunks
    SRCWF = din("SRCWF", [128, NWIN, KT], F32)
    OUTT = nc.dram_tensor("OUTT", [NPC, OUT], F32, kind="ExternalOutput")

    with tile.TileContext(nc) as tc:
        with tc.tile_pool(name="const", bufs=1) as cpool:
            e4 = cpool.tile([4, 1], F32)
            nc.gpsimd.memset(e4[:], 1.0)
            slp = cpool.tile([128, 1], F32)
            nc.gpsimd.memset(slp[:], SLOPE)
            iota_bf = cpool.tile([128, 128], BF16)
            nc.gpsimd.iota(iota_bf[:], pattern=[[1, 128]], base=0,
                           channel_multiplier=0,
                           allow_small_or_imprecise_dtypes=True)
            srcwf = cpool.tile([128, NWIN, KT], F32)
            nc.sync.dma_start(srcwf[:], SRCWF[:])
            hall = cpool.tile([128, NWIN, OUT], F32)
            with (
                tc.tile_pool(name="w", bufs=3) as wp,
                tc.tile_pool(name="wv", bufs=6) as wv,
                tc.tile_pool(name="ws_ps", bufs=2, space="PSUM") as wsp,
                tc.tile_pool(name="wn_ps", bufs=2, space="PSUM") as wnp,
                tc.tile_pool(name="wd_ps", bufs=2, space="PSUM") as wdp,
            ):
                for w in range(NWIN):
                    hg = wp.tile([128, KT, OUT], BF16, tag="hg")
                    nc.sync.dma_start(hg[:], HG[w])
                    cm2 = wp.tile([4, NS], F32, tag="cm2")
                    nc.sync.dma_start(cm2[:], CM2[w])
                    s3 = wp.tile([128, KT, 128], BF16, tag="s3")
                    nc.sync.dma_start(s3[:, :NDMA3, :], S3D[w])
                    # build remaining one-hot chunks on DVE / Pool
                    for c in range(NDMA3, KT):
                        if c % 2 == 0:
                            nc.vector.tensor_scalar(
                                s3[:, c, :], iota_bf[:],
                                srcwf[:, w, c : c + 1], None,
                                op0=OP.is_equal)
                        else:
                            nc.gpsimd.tensor_scalar(
                                s3[:, c, :], iota_bf[:],
                                srcwf[:, w, c : c + 1], None,
                                op0=OP.is_equal)
                    s1_ps = wsp.tile([128, KT], F32, tag="s1")
                    for c in range(KT):
                        nc.tensor.matmul(s1_ps[:, c : c + 1],
                                         lhsT=cm2[:, 128 * c : 128 * (c + 1)],
                                         rhs=e4[:], start=True, stop=True)
                    # w1 = max(exp(s), exp(0.01 s))
                    ex1 = wv.tile([128, KT], BF16, tag="ex1")
                    nc.scalar.activation(ex1[:], s1_ps[:], AF.Exp)
                    ex2 = wv.tile([128, KT], BF16, tag="ex2")
                    nc.scalar.activation(ex2[:], s1_ps[:], AF.Exp,
                                         scale=slp[:])
                    w1 = wv.tile([128, KT], BF16, tag="w1")
                    nc.vector.tensor_tensor(out=w1[:], in0=ex1[:],
                                            in1=ex2[:], op=OP.max)
                    V1 = wv.tile([128, KT, OUT], BF16, tag="V1")
                    h3 = KT // 3
                    nc.vector.tensor_tensor(
                        out=V1[:, : 2 * h3, :], in0=hg[:, : 2 * h3, :],
                        in1=w1[:, : 2 * h3].to_broadcast(
                            [128, 2 * h3, OUT]), op=OP.mult)
                    nc.gpsimd.tensor_tensor(
                        out=V1[:, 2 * h3 :, :], in0=hg[:, 2 * h3 :, :],
                        in1=w1[:, 2 * h3 :].to_broadcast(
                            [128, KT - 2 * h3, OUT]),
                        op=OP.mult)
                    num_ps = wnp.tile([128, OUT], F32, tag="num")
                    den_ps = wdp.tile([128, 1], F32, tag="den")
                    for c in range(KT):
                        nc.tensor.matmul(num_ps[:], lhsT=s3[:, c, :],
                                         rhs=V1[:, c, :],
                                         start=(c == 0), stop=(c == KT - 1))
                        nc.tensor.matmul(den_ps[:], lhsT=s3[:, c, :],
                                         rhs=w1[:, c : c + 1],
                                         start=(c == 0), stop=(c == KT - 1))
                    den = wv.tile([128, 1], F32, tag="dens")
                    nc.vector.tensor_scalar(den[:], den_ps[:], 1e-16, None,
                                            op0=OP.add)
                    nc.vector.reciprocal(den[:], den[:])
                    h2 = wv.tile([128, OUT], F32, tag="h2")
                    nc.vector.tensor_scalar(h2[:], num_ps[:], den[:], None,
                                            op0=OP.mult)
                    m0 = wv.tile([128, OUT], F32, tag="m0")
                    nc.vector.tensor_scalar_min(m0[:], h2[:], 0.0)
                    nc.scalar.activation(m0[:], m0[:], AF.Exp)
                    r0 = wv.tile([128, OUT], F32, tag="r0")
                    nc.vector.tensor_scalar(r0[:], h2[:], 0.0, -1.0,
                                            op0=OP.max, op1=OP.add)
                    nc.vector.tensor_tensor(out=hall[:, w, :], in0=m0[:],
                                            in1=r0[:], op=OP.add)
            with tc.tile_pool(name="fin", bufs=1) as fin:
                ex = fin.tile([128, NWIN, OUT], F32)
                nc.scalar.activation(ex[:], hall[:], AF.Exp)
                sm = fin.tile([128, NWIN], F32)
                nc.vector.tensor_reduce(sm[:], ex[:], axis=AX.X, op=OP.add)
                nc.scalar.activation(sm[:], sm[:], AF.Ln)
                res = fin.tile([128, NWIN, OUT], F32)
                nc.vector.tensor_tensor(
                    out=res[:], in0=hall[:],
                    in1=sm[:].to_broadcast([128, NWIN, OUT]), op=OP.subtract)
                nc.sync.dma_start(
                    OUTT[:].rearrange("(w p) f -> p w f", p=128), res[:])
    nc.compile()
    return nc


# ------------------------------------------------------------------ driver


def kernel(X, edge_attr, w_node, b_node, g_node, beta_node,
           w_edge, b_edge, g_edge, beta_edge,
           gat_W, gat_a, out_W, out_a,
           edge_index, matched_car_infra_nodes):
    import ml_dtypes
    import time as _time

    bf = lambda a: np.ascontiguousarray(np.asarray(a, np.float32)).astype(
        ml_dtypes.bfloat16)
    f32 = lambda a: np.ascontiguousarray(np.asarray(a, np.float32))

    X = f32(X)
    ea = f32(edge_attr)
    w_node = f32(w_node); b_node = f32(b_node); g_node = f32(g_node)
    beta_node = f32(beta_node)
    w_edge = f32(w_edge); b_edge = f32(b_edge); g_edge = f32(g_edge)
    beta_edge = f32(beta_edge)
    gW = f32(gat_W); ga = f32(gat_a); oW = f32(out_W); oa = f32(out_a)
    assert np.abs(beta_node).max() < 1e-6 and np.abs(beta_edge).max() < 1e-6

    per_core, pnode, origin, KT, srcw_of, ptgt = _prep(edge_index)
    NS = KT * 128
    NCHE = NWIN * KT
    NCHE4 = ((NCHE + 15) // 16) * 16

    # ---- LN-folded weights (centered + Cholesky u-columns)
    def fold(Wb, bb, g, kdim):
        Wfull = np.concatenate([Wb, bb[None, :]], 0)          # [k, 64]
        m = Wfull.mean(axis=1)                                 # [k]
        Wc = Wfull - m[:, None]
        M = Wc @ Wc.T + 1e-10 * np.eye(kdim)
        B = np.linalg.cholesky(M) / np.sqrt(EMB)
        return np.concatenate([Wc * g[None, :], B], 1)         # [k, 64+k]

    WNC = bf(fold(w_node, b_node, g_node, NODE_DIM + 1))
    WEC = bf(fold(w_edge, b_edge, g_edge, EA_DIM + 1))
    WAB = np.zeros((EMB, 16), np.float32)
    for i in range(HEADS):
        WAB[:, i] = gW[i] @ ga[i, :OUT]
        WAB[:, 8 + i] = gW[i] @ ga[i, OUT : 2 * OUT]
    WAB = bf(WAB)
    AE9 = np.zeros((EMB, 16), np.float32)
    for i in range(HEADS):
        AE9[:, i] = ga[i, 2 * OUT :]
    AE9[:, 8] = oa[2 * OUT :]
    A2 = np.zeros((2, 128, 16), np.float32)
    A2[0, :EMB] = AE9
    A2[1, EMB:] = AE9
    AE9 = bf(A2.transpose(1, 0, 2))

    # ---- launch 1 inputs
    Xp = np.zeros((NPN, NODE_DIM + 1), np.float32)
    valid = origin >= 0
    Xp[valid, :NODE_DIM] = X[origin[valid]]
    Xp[:, NODE_DIM] = 1.0
    src = np.asarray(edge_index[0]).astype(np.int64)

    in_maps1 = []
    for k in range(NCORES):
        eslot = per_core[k]                                    # [NWIN, NS]
        eat = np.zeros((NCHE4 * 128, EA_DIM + 1), np.float32)
        es = eslot.reshape(-1)
        m = es >= 0
        eat[: NS * NWIN][m, :EA_DIM] = ea[es[m]]
        eat[: NS * NWIN][m, EA_DIM] = 1.0
        in_maps1.append(dict(
            XT17=bf(Xp[k * NPC : (k + 1) * NPC].T),
            WNC=WNC, WAB=WAB, AE9=AE9,
            EAT9=bf(eat.T), WEC=WEC))

    nc1 = _build_launch1(NCHE4)
    kernel.nc1 = nc1
    _t = _time.perf_counter()
    res1 = run_bass_kernel_spmd(nc1, in_maps1, core_ids=list(range(NCORES)))
    kernel.wall1 = _time.perf_counter() - _t

    # ---- host: assemble tables, gather per-slot inputs for launch 2
    XF = np.zeros((NPN, EMB), ml_dtypes.bfloat16)
    ADF = np.zeros((NPN, 16), np.float32)
    ESCF = []
    for k in range(NCORES):
        XF[k * NPC : (k + 1) * NPC] = res1.results[k]["XO"]
        ADF[k * NPC : (k + 1) * NPC] = res1.results[k]["AD"]
        # ESC9 [128, NCHE4, 9] -> slot-major [NWIN, NS, 9]
        e9 = np.asarray(res1.results[k]["ESC9"], np.float32)[:, :NCHE, :]
        e9 = e9.transpose(1, 0, 2).reshape(NWIN, NS, 9)
        ESCF.append(e9)

    # one-hot S3 per core (shared by launches 2 and 3)
    in_maps2 = []
    s3_cores = []
    for k in range(NCORES):
        eslot = per_core[k]
        es = eslot.reshape(NWIN, NS)
        m = es >= 0
        tgtrow = np.zeros((NWIN, NS), np.int64)
        tgtrow[m] = ptgt[es[m]]
        srcw = np.full((NWIN, NS), -1, np.int64)
        srcw[m] = srcw_of[es[m]]

        XGk = np.zeros((NWIN, NS, EMB), ml_dtypes.bfloat16)
        XGk[m] = XF[tgtrow[m]]
        CMTk = np.zeros((NWIN, 24, NS), np.float32)
        CMTk[:, 0:8, :] = ESCF[k][:, :, 0:8].transpose(0, 2, 1)
        srcrow_k = np.zeros((NWIN, NS), np.int64)
        # src row = core base + win*128 + srcw
        wid = np.arange(NWIN)[:, None]
        srcrow_k[m] = (k * NPC + (wid + np.zeros_like(srcw))[m] * 128
                       + srcw[m])
        asrc = np.zeros((NWIN, NS, 8), np.float32)
        asrc[m] = ADF[srcrow_k[m], 0:8]
        atgt = np.zeros((NWIN, NS, 8), np.float32)
        atgt[m] = ADF[tgtrow[m], 8:16]
        CMTk[:, 8:16, :] = asrc.transpose(0, 2, 1)
        CMTk[:, 16:24, :] = atgt.transpose(0, 2, 1)

        S3k = np.zeros((NWIN, NS, 128), ml_dtypes.bfloat16)
        ww, ss = np.nonzero(m)
        S3k[ww, ss, srcw[ww, ss]] = 1.0
        S3k = S3k.reshape(NWIN, KT, 128, 128).transpose(0, 2, 1, 3)
        s3_cores.append(np.ascontiguousarray(S3k))

        WB4 = np.zeros((128, 4, 128), np.float32)
        for j in range(4):
            for il in range(2):
                WB4[64 * il : 64 * il + 64, j,
                    64 * il : 64 * il + 64] = gW[2 * j + il]
        E24 = np.zeros((24, 8), np.float32)
        for i in range(8):
            E24[i, i] = 1.0
            E24[8 + i, i] = 1.0
            E24[16 + i, i] = 1.0
        OWC = np.zeros((512, 66), np.float32)
        OWC[:, 0:64] = oW
        OWC[:, 64] = oW @ oa[:OUT]
        OWC[:, 65] = oW @ oa[OUT : 2 * OUT]
        in_maps2.append(dict(
            XG=_slotmaj(XGk, KT, EMB),
            CMT=CMTk,
            S3H=s3_cores[k],
            E24=E24,
            OWC=bf(np.ascontiguousarray(
                OWC.reshape(4, 128, 66).transpose(1, 0, 2))),
            WB4=bf(WB4),
        ))

    nc2 = _build_launch2(KT)
    kernel.nc2 = nc2
    _t = _time.perf_counter()
    res2 = run_bass_kernel_spmd(nc2, in_maps2, core_ids=list(range(NCORES)))
    kernel.wall2 = _time.perf_counter() - _t

    # ---- host: assemble h_out table, gather for launch 3
    HF = np.zeros((NPN, 66), np.float32)
    for k in range(NCORES):
        HF[k * NPC : (k + 1) * NPC] = res2.results[k]["HOUTS"]
    HFb = HF[:, 0:64].astype(ml_dtypes.bfloat16)

    in_maps3 = []
    for k in range(NCORES):
        eslot = per_core[k]
        es = eslot.reshape(NWIN, NS)
        m = es >= 0
        tgtrow = np.zeros((NWIN, NS), np.int64)
        tgtrow[m] = ptgt[es[m]]
        srcw = np.full((NWIN, NS), -1, np.int64)
        srcw[m] = srcw_of[es[m]]
        wid = np.arange(NWIN)[:, None]
        srcrow_k = np.zeros((NWIN, NS), np.int64)
        srcrow_k[m] = (k * NPC + (wid + np.zeros_like(srcw))[m] * 128
                       + srcw[m])

        HGk = np.zeros((NWIN, NS, OUT), ml_dtypes.bfloat16)
        HGk[m] = HFb[tgtrow[m]]
        CM2k = np.zeros((NWIN, 4, NS), np.float32)
        CM2k[:, 0, :] = ESCF[k][:, :, 8]
        a_s = np.zeros((NWIN, NS), np.float32)
        a_s[m] = HF[srcrow_k[m], 64]
        a_t = np.zeros((NWIN, NS), np.float32)
        a_t[m] = HF[tgtrow[m], 65]
        CM2k[:, 1, :] = a_s
        CM2k[:, 2, :] = a_t
        srcwf_f = srcw.reshape(NWIN, KT, 128).transpose(2, 0, 1).astype(
            np.float32)
        in_maps3.append(dict(
            HG=_slotmaj(HGk, KT, OUT),
            CM2=CM2k,
            S3D=np.ascontiguousarray(s3_cores[k][:, :, :NDMA3, :]),
            SRCWF=np.ascontiguousarray(srcwf_f)))

    nc3 = _build_launch3(KT)
    kernel.nc3 = nc3
    _t = _time.perf_counter()
    res3 = run_bass_kernel_spmd(nc3, in_maps3, core_ids=list(range(NCORES)))
    kernel.wall3 = _time.perf_counter() - _t

    outp = np.zeros((NPN, OUT), np.float32)
    for k in range(NCORES):
        outp[k * NPC : (k + 1) * NPC] = res3.results[k]["OUTT"]
    out = np.zeros((N, OUT), np.float32)
    valid = origin >= 0
    out[origin[valid]] = outp[valid]
    return out


def _slotmaj(A, KT, F):
    """[NWIN, NS, F] with slot s=(c*128+p) -> [NWIN, 128, KT, F]."""
    NW = A.shape[0]
    return np.ascontiguousarray(
        A.reshape(NW, KT, 128, F).transpose(0, 2, 1, 3))
